# revision 33
# baseline (speedup 1.0000x reference)
"""LocalVarianceMap Trainium2 kernel.

reference:
  lum  = mean over channel of x            (B,1,H,W)
  mean = 7x7 'same' box mean of lum ; sqm = same of lum^2
  out  = sqm - mean^2

Full input x: (16, 3, 1024, 1024) fp32. Data-parallel over batch:
8 NeuronCores x 2 images each. HBM traffic/core = 33.6 MB -> ~93 us
at the 360 GB/s DMA-bus share, which is the target wall.

Per-core pipeline per 128-row tile (partition=h, free=w). Let
s = x0+x1+x2 (channel sum, = 3*lum); all intermediate tiles fp16:

  SP  : 3 HWDGE loads (x0,x1,x2 rows)           [4KB packets]
  Pool: l01 = x0+x1                (fp32)
  DVE : s   = l01+x2               (fp16, stt)
  ACT : sq  = s^2                  (fp16)
  DVE : h1  = 7-col sliding sum of s  (one tensor_tensor_scan, fp32
        carried state, fp16 out), h2 = same of sq
  PE  : S1  = Wband^T @ h1  (2 fp16 matmuls, vertical 7-band sum)
  ACT : m2  = Square(S1/147)       (fp16) == mean^2
  PE  : S2  = Wband^T @ h2 - 441*I @ m2  (4 fp16 matmuls, PSUM accum)
  ACT : V   = Copy(S2/441)         (fp32) == var
  SP  : HWDGE store V

fp16 is safe here: the scan keeps its running state in fp32 (only the
stored outputs round), and all other fp16 roundings are per-element
(bounded, no accumulation across the row).

Vertical tiling overlaps input tiles by 6 rows so each output tile's
vertical halo lives inside one K<=128 banded matmul.
"""

import sys

if "/opt/trn_rl_repo" not in sys.path:
    sys.path.insert(0, "/opt/trn_rl_repo")

import numpy as np
from contextlib import ExitStack

import concourse.bass as bass
import concourse.bacc as bacc
import concourse.tile as tile
from concourse import mybir

H = 1024
W = 1024
C = 3
PER_CORE_B = 2
N_CORES = 8
K7 = 7
PADL = 8                 # left zero pad of s/sq tiles (scan telescoping);
                         # even, so packed-f16 writes at PADL stay 4B-aligned
SW = PADL + W + 4        # fp16 row-tile width (1036; used cols 0..1034)
SCAN_N = W + 4           # even scan width (N=1028): the DVE scan runs 1 cyc/elem
                         # at even N>512, 2 cyc/elem otherwise; h[1028] is unused
HW_ = 1024               # scan data width

f32 = mybir.dt.float32
f16 = mybir.dt.float16
bf16 = mybir.dt.bfloat16
ADD = mybir.AluOpType.add
SUB = mybir.AluOpType.subtract
MUL = mybir.AluOpType.mult
Square = mybir.ActivationFunctionType.Square
Copy = mybir.ActivationFunctionType.Copy


def _tiles():
    # last 48-row tiles of both images share one 96-partition tile (a full
    # tile's scans cost the same regardless of row count)
    # combined tile first: smallest load (1.125 MB) -> fastest pipeline fill
    specs = [dict(b=-1, r0=976, nr=96, K=96, M=90, out_r0=979, w=3)]
    for b in range(PER_CORE_B):
        specs.append(dict(b=b, r0=0, nr=128, K=128, M=125, out_r0=0, w=0))
        for t in range(1, 8):
            specs.append(
                dict(b=b, r0=122 * t, nr=128, K=128, M=122, out_r0=122 * t + 3, w=1)
            )
    return specs


def band_weights() -> tuple[np.ndarray, np.ndarray]:
    """(fp16 [128,512]: W0|Wmid|Wlast|Wcomb for the mean path,
    bf16 [128,640]: same four bands | -441*I for the squares path)."""
    wb = np.zeros((128, 5 * 128), np.float32)
    for m in range(125):
        for k in range(max(m - 3, 0), m + 4):
            wb[k, m] = 1.0
    for m in range(122):
        for k in range(m, m + 7):
            wb[k, 128 + m] = 1.0
    for m in range(45):
        for k in range(m, min(m + 7, 48)):
            wb[k, 256 + m] = 1.0
    # block 3: both images' bottom-edge tiles, block-diagonal on partitions
    for m in range(45):
        for k in range(m, min(m + 7, 48)):
            wb[k, 384 + m] = 1.0
            wb[48 + k, 384 + 45 + m] = 1.0
    for m in range(128):
        wb[m, 512 + m] = -441.0
    import ml_dtypes

    return wb[:, 0:512].astype(np.float16), wb.astype(ml_dtypes.bfloat16)


def build_nc(finalize: bool = True) -> bass.Bass:
    nc = bacc.Bacc("TRN2", target_bir_lowering=False)

    x = nc.dram_tensor("x", [PER_CORE_B, C, H, W], f32, kind="ExternalInput")
    wbt = nc.dram_tensor("wb", [128, 4 * 128], f16, kind="ExternalInput")
    wbbt = nc.dram_tensor("wbb", [128, 5 * 128], bf16, kind="ExternalInput")
    y = nc.dram_tensor("y", [PER_CORE_B, 1, H, W], f16, kind="ExternalOutput")

    inv147 = float(np.float32(1.0) / np.float32(147.0))
    inv441 = float(np.float32(1.0) / np.float32(441.0))

    specs = _tiles()
    NT = len(specs)

    NXIN, NL01, NLUM, NSQ, NH1, NH2, NM2, NV = 6, 4, 4, 4, 3, 4, 3, 3

    with tile.TileContext(nc) as tc, ExitStack() as ctx:
        cpool = ctx.enter_context(tc.tile_pool(name="const", bufs=1))
        p1pool = ctx.enter_context(tc.tile_pool(name="ps1", bufs=2, space="PSUM"))
        p2pool = ctx.enter_context(tc.tile_pool(name="ps2", bufs=2, space="PSUM"))

        WB = cpool.tile([128, 4 * 128], f16)
        nc.sync.dma_start(out=WB[:], in_=wbt[:, :])
        WBB = cpool.tile([128, 5 * 128], bf16)
        nc.sync.dma_start(out=WBB[:], in_=wbbt[:, :])

        def ring(n, shape, dt, nm):
            return [cpool.tile(shape, dt, tag=f"{nm}{i}", name=f"{nm}{i}") for i in range(n)]

        xin_ring = ring(NXIN, [128, 3 * W], f32, "xin")
        l01_ring = ring(NL01, [128, W], f16, "l01")
        xc_ring = ring(NL01, [128, 3 * W], f16, "xc")
        l01f_ring = ring(3, [128, W], f32, "l01f")  # fill fast path only
        lum_ring = ring(NLUM, [128, SW], f16, "lum")
        # squares path in bf16: lum^2 underflows into f16 subnormals
        # (~0.4% of elements), which stall the DVE scan ~2x; bf16 has no
        # reachable subnormal range here.
        sq_ring = ring(NSQ, [128, SW], bf16, "sq")
        h1_ring = ring(NH1, [128, SCAN_N + 4], f16, "h1")
        h2_ring = ring(NH2, [128, SCAN_N + 4], bf16, "h2")
        m2_ring = ring(NM2, [128, W], bf16, "m2")
        v_ring = ring(NV, [128, W], f16, "v")  # f16 store halves DMA bytes; ~2.4e-4 rel rounding

        # zero the horizontal halo pads once (tiles are reused; the image
        # region is rewritten every tile, pads stay zero)
        for t_ in lum_ring + sq_ring:
            nc.gpsimd.memset(t_[:, 0:PADL], 0.0)
            nc.gpsimd.memset(t_[:, PADL + W : SW], 0.0)

        def s0_load(t):
            sp = specs[t]
            xin = xin_ring[t % NXIN]
            if sp["b"] < 0:  # combined bottom tiles: both images, 48 rows each
                for b in range(PER_CORE_B):
                    for c in range(C):
                        nc.sync.dma_start(
                            out=xin[48 * b : 48 * b + 48, c * W : (c + 1) * W],
                            in_=x[b, c, sp["r0"] : sp["r0"] + 48, :],
                        )
                return
            for c in range(C):
                # during pipeline fill, split across both HWDGE queues
                eng = nc.scalar if (t < 3 and c == 1) else nc.sync
                eng.dma_start(
                    out=xin[0 : sp["nr"], c * W : (c + 1) * W],
                    in_=x[sp["b"], c, sp["r0"] : sp["r0"] + sp["nr"], :],
                )

        def s0c_conv(t):
            # all 3 channels -> f16 in one wide ACT op; both channel adds
            # then run in the DVE 2x packed-f16 mode
            if t < 3:
                return  # fill fast path adds in f32 straight from xin
            sp = specs[t]
            nr = sp["nr"]
            xin = xin_ring[t % NXIN]
            nc.scalar.activation(
                xc_ring[t % NL01][0:nr, :], xin[0:nr, :], Copy
            )

        def s1_l01(t):
            # adds on DVE, not Pool: concurrent Pool(gpsimd) activity slows
            # DVE scans ~2x (measured), and DVE is the critical engine
            sp = specs[t]
            nr = sp["nr"]
            if t < 3:
                xin = xin_ring[t % NXIN]
                nc.vector.tensor_tensor(
                    l01f_ring[t][0:nr, :],
                    xin[0:nr, 0:W],
                    xin[0:nr, W : 2 * W],
                    ADD,
                )
                return
            xc = xc_ring[t % NL01]
            nc.vector.tensor_tensor(
                l01_ring[t % NL01][0:nr, :], xc[0:nr, 0:W], xc[0:nr, W : 2 * W], ADD
            )

        def s2_lum(t):
            sp = specs[t]
            nr = sp["nr"]
            if t < 3:
                xin = xin_ring[t % NXIN]
                nc.vector.tensor_tensor(
                    lum_ring[t % NLUM][0:nr, PADL : PADL + W],
                    l01f_ring[t][0:nr, :],
                    xin[0:nr, 2 * W : 3 * W],
                    ADD,
                )
                return
            nc.vector.tensor_tensor(
                lum_ring[t % NLUM][0:nr, PADL : PADL + W],
                l01_ring[t % NL01][0:nr, :],
                xc_ring[t % NL01][0:nr, 2 * W : 3 * W],
                ADD,
            )

        def s3_sq(t):
            sp = specs[t]
            nr = sp["nr"]
            lum = lum_ring[t % NLUM]
            nc.scalar.activation(
                sq_ring[t % NSQ][0:nr, PADL : PADL + W],
                lum[0:nr, PADL : PADL + W],
                Square,
            )

        def s4_scans(t):
            sp = specs[t]
            nr = sp["nr"]
            # out[c] = sum_{v=c+1..c+7} src[v]; anchor is exact because
            # src[0..7] are zero pad. matmul rhs for output col j reads
            # out[j+4] (even offsets)
            for src_ring, nsrc, dst_ring, ndst in (
                (lum_ring, NLUM, h1_ring, NH1),
                (sq_ring, NSQ, h2_ring, NH2),
            ):
                src = src_ring[t % nsrc]
                dst = dst_ring[t % ndst]
                nc.vector.tensor_tensor_scan(
                    out=dst[0:nr, 0:SCAN_N],
                    data0=src[0:nr, 7 : 7 + SCAN_N],
                    data1=src[0:nr, 0:SCAN_N],
                    initial=0.0,
                    op0=ADD,
                    op1=SUB,
                )

        S1s, S2s = {}, {}

        def s5_mm1(t):
            sp = specs[t]
            K, M, w = sp["K"], sp["M"], sp["w"]
            h1 = h1_ring[t % NH1]
            S1 = p1pool.tile([128, W], f32, tag="S1", name=f"S1_{t}")
            for half in range(2):
                nc.tensor.matmul(
                    S1[0:M, 512 * half : 512 * half + 512],
                    WB[0:K, 128 * w : 128 * w + M],
                    h1[0:K, 4 + 512 * half : 516 + 512 * half],
                    start=True,
                    stop=True,
                )
            S1s[t] = S1

        def s6_m2(t):
            sp = specs[t]
            M = sp["M"]
            nc.scalar.activation(
                m2_ring[t % NM2][0:M, :], S1s.pop(t)[0:M, :], Square, scale=inv147
            )

        def s7_mm2(t):
            sp = specs[t]
            K, M, w = sp["K"], sp["M"], sp["w"]
            h2 = h2_ring[t % NH2]
            m2 = m2_ring[t % NM2]
            S2 = p2pool.tile([128, W], f32, tag="S2", name=f"S2_{t}")
            for half in range(2):
                c0 = 512 * half
                nc.tensor.matmul(
                    S2[0:M, c0 : c0 + 512],
                    WBB[0:K, 128 * w : 128 * w + M],
                    h2[0:K, 4 + c0 : 516 + c0],
                    start=True,
                    stop=False,
                    skip_group_check=True,
                )
            for half in range(2):
                c0 = 512 * half
                nc.tensor.matmul(
                    S2[0:M, c0 : c0 + 512],
                    WBB[0:M, 512 : 512 + M],
                    m2[0:M, c0 : c0 + 512],
                    start=False,
                    stop=True,
                    skip_group_check=True,
                )
            S2s[t] = S2

        def s8_v(t):
            sp = specs[t]
            M = sp["M"]
            nc.scalar.activation(
                v_ring[t % NV][0:M, :], S2s.pop(t)[0:M, :], Copy, scale=inv441
            )

        def s9_store(t):
            # SWDGE: software descriptors spread the store across all 16 SDMA
            # engines; an HWDGE store ring pins to 2 engines (22.5 GB/s each).
            sp = specs[t]
            M = sp["M"]
            if sp["b"] < 0:
                # tiny final stores: HWDGE (sync+scalar) beats SWDGE generation
                for b in range(PER_CORE_B):
                    eng = nc.sync if b == 0 else nc.scalar
                    eng.dma_start(
                        out=y[b, 0, sp["out_r0"] : sp["out_r0"] + 45, :],
                        in_=v_ring[t % NV][45 * b : 45 * b + 45, :],
                    )
                return
            nc.gpsimd.dma_start(
                out=y[sp["b"], 0, sp["out_r0"] : sp["out_r0"] + M, :],
                in_=v_ring[t % NV][0:M, :],
            )

        for i in range(NT + 8):
            if 7 <= i < NT + 7:
                s9_store(i - 7)
            if i < NT:
                s0_load(i)
            if 6 <= i < NT + 6:
                s7_mm2(i - 6)
            if 6 <= i < NT + 6:
                s8_v(i - 6)
            if 5 <= i < NT + 5:
                s6_m2(i - 5)
            if 3 <= i < NT + 3:
                s3_sq(i - 3)
            if 4 <= i < NT + 4:
                s4_scans(i - 4)
            if 4 <= i < NT + 4:
                s5_mm1(i - 4)
            if 2 <= i < NT + 2:
                s1_l01(i - 2)
            if 2 <= i < NT + 2:
                s2_lum(i - 2)
            if 1 <= i < NT + 1:
                s0c_conv(i - 1)

    if finalize:
        nc.finalize()
    return nc


def kernel(x, kernel_size):
    assert int(kernel_size) == K7
    x = np.ascontiguousarray(np.asarray(x, dtype=np.float32))
    B = x.shape[0]
    assert x.shape == (B, C, H, W) and B == PER_CORE_B * N_CORES

    from concourse.bass_utils import run_bass_kernel_spmd

    nc = build_nc()
    wb, wbb = band_weights()
    in_maps = [
        {"x": x[i * PER_CORE_B : (i + 1) * PER_CORE_B], "wb": wb, "wbb": wbb}
        for i in range(N_CORES)
    ]
    res = run_bass_kernel_spmd(nc, in_maps, list(range(N_CORES)))
    y = np.concatenate(
        [np.asarray(res.results[i]["y"], dtype=np.float32) for i in range(N_CORES)],
        axis=0,
    )
    return y


# revision 34
# speedup vs baseline: 1.0490x; 1.0490x over previous
"""LocalVarianceMap Trainium2 kernel.

reference:
  lum  = mean over channel of x            (B,1,H,W)
  mean = 7x7 'same' box mean of lum ; sqm = same of lum^2
  out  = sqm - mean^2

Full input x: (16, 3, 1024, 1024) fp32. Data-parallel over batch:
8 NeuronCores x 2 images each. HBM traffic/core = 33.6 MB -> ~93 us
at the 360 GB/s DMA-bus share, which is the target wall.

Per-core pipeline per 128-row tile (partition=h, free=w). Let
s = x0+x1+x2 (channel sum, = 3*lum); all intermediate tiles fp16:

  SP  : 3 HWDGE loads (x0,x1,x2 rows)           [4KB packets]
  Pool: l01 = x0+x1                (fp32)
  DVE : s   = l01+x2               (fp16, stt)
  ACT : sq  = s^2                  (fp16)
  DVE : h1  = 7-col sliding sum of s  (one tensor_tensor_scan, fp32
        carried state, fp16 out), h2 = same of sq
  PE  : S1  = Wband^T @ h1  (2 fp16 matmuls, vertical 7-band sum)
  ACT : m2  = Square(S1/147)       (fp16) == mean^2
  PE  : S2  = Wband^T @ h2 - 441*I @ m2  (4 fp16 matmuls, PSUM accum)
  ACT : V   = Copy(S2/441)         (fp32) == var
  SP  : HWDGE store V

fp16 is safe here: the scan keeps its running state in fp32 (only the
stored outputs round), and all other fp16 roundings are per-element
(bounded, no accumulation across the row).

Vertical tiling overlaps input tiles by 6 rows so each output tile's
vertical halo lives inside one K<=128 banded matmul.
"""

import sys

if "/opt/trn_rl_repo" not in sys.path:
    sys.path.insert(0, "/opt/trn_rl_repo")

import numpy as np
from contextlib import ExitStack

import concourse.bass as bass
import concourse.bacc as bacc
import concourse.tile as tile
from concourse import mybir

H = 1024
W = 1024
C = 3
PER_CORE_B = 2
N_CORES = 8
K7 = 7
PADL = 8                 # left zero pad of s/sq tiles (scan telescoping);
                         # even, so packed-f16 writes at PADL stay 4B-aligned
SW = PADL + W + 4        # fp16 row-tile width (1036; used cols 0..1034)
SCAN_N = W + 4           # even scan width (N=1028): the DVE scan runs 1 cyc/elem
                         # at even N>512, 2 cyc/elem otherwise; h[1028] is unused
HW_ = 1024               # scan data width

f32 = mybir.dt.float32
f16 = mybir.dt.float16
bf16 = mybir.dt.bfloat16
ADD = mybir.AluOpType.add
SUB = mybir.AluOpType.subtract
MUL = mybir.AluOpType.mult
Square = mybir.ActivationFunctionType.Square
Copy = mybir.ActivationFunctionType.Copy


def _tiles():
    # last 48-row tiles of both images share one 96-partition tile (a full
    # tile's scans cost the same regardless of row count)
    specs = []
    for b in range(PER_CORE_B):
        specs.append(dict(b=b, r0=0, nr=128, K=128, M=125, out_r0=0, w=0))
        for t in range(1, 8):
            specs.append(
                dict(b=b, r0=122 * t, nr=128, K=128, M=122, out_r0=122 * t + 3, w=1)
            )
    specs.append(dict(b=-1, r0=976, nr=96, K=96, M=90, out_r0=979, w=3))
    return specs


def band_weights() -> tuple[np.ndarray, np.ndarray]:
    """(fp16 [128,512]: W0|Wmid|Wlast|Wcomb for the mean path,
    bf16 [128,640]: same four bands | -441*I for the squares path)."""
    wb = np.zeros((128, 5 * 128), np.float32)
    for m in range(125):
        for k in range(max(m - 3, 0), m + 4):
            wb[k, m] = 1.0
    for m in range(122):
        for k in range(m, m + 7):
            wb[k, 128 + m] = 1.0
    for m in range(45):
        for k in range(m, min(m + 7, 48)):
            wb[k, 256 + m] = 1.0
    # block 3: both images' bottom-edge tiles, block-diagonal on partitions
    for m in range(45):
        for k in range(m, min(m + 7, 48)):
            wb[k, 384 + m] = 1.0
            wb[48 + k, 384 + 45 + m] = 1.0
    for m in range(128):
        wb[m, 512 + m] = -441.0
    import ml_dtypes

    return wb[:, 0:512].astype(np.float16), wb.astype(ml_dtypes.bfloat16)


def build_nc(finalize: bool = True) -> bass.Bass:
    nc = bacc.Bacc("TRN2", target_bir_lowering=False)

    x = nc.dram_tensor("x", [PER_CORE_B, C, H, W], f32, kind="ExternalInput")
    wbt = nc.dram_tensor("wb", [128, 4 * 128], f16, kind="ExternalInput")
    wbbt = nc.dram_tensor("wbb", [128, 5 * 128], bf16, kind="ExternalInput")
    y = nc.dram_tensor("y", [PER_CORE_B, 1, H, W], f16, kind="ExternalOutput")

    inv147 = float(np.float32(1.0) / np.float32(147.0))
    inv441 = float(np.float32(1.0) / np.float32(441.0))

    specs = _tiles()
    NT = len(specs)

    NXIN, NL01, NLUM, NSQ, NH1, NH2, NM2, NV = 6, 4, 4, 4, 3, 4, 3, 3

    with tile.TileContext(nc) as tc, ExitStack() as ctx:
        cpool = ctx.enter_context(tc.tile_pool(name="const", bufs=1))
        p1pool = ctx.enter_context(tc.tile_pool(name="ps1", bufs=2, space="PSUM"))
        p2pool = ctx.enter_context(tc.tile_pool(name="ps2", bufs=2, space="PSUM"))

        WB = cpool.tile([128, 4 * 128], f16)
        nc.sync.dma_start(out=WB[:], in_=wbt[:, :])
        WBB = cpool.tile([128, 5 * 128], bf16)
        nc.sync.dma_start(out=WBB[:], in_=wbbt[:, :])

        def ring(n, shape, dt, nm):
            return [cpool.tile(shape, dt, tag=f"{nm}{i}", name=f"{nm}{i}") for i in range(n)]

        xin_ring = ring(NXIN, [128, 3 * W], f32, "xin")
        l01_ring = ring(NL01, [128, W], f16, "l01")
        xc_ring = ring(NL01, [128, 3 * W], f16, "xc")
        l01f_ring = ring(2, [128, W], f32, "l01f")  # fill fast path only
        lum_ring = ring(NLUM, [128, SW], f16, "lum")
        # squares path in bf16: lum^2 underflows into f16 subnormals
        # (~0.4% of elements), which stall the DVE scan ~2x; bf16 has no
        # reachable subnormal range here.
        sq_ring = ring(NSQ, [128, SW], bf16, "sq")
        h1_ring = ring(NH1, [128, SCAN_N + 4], f16, "h1")
        h2_ring = ring(NH2, [128, SCAN_N + 4], bf16, "h2")
        m2_ring = ring(NM2, [128, W], bf16, "m2")
        v_ring = ring(NV, [128, W], f16, "v")  # f16 store halves DMA bytes; ~2.4e-4 rel rounding

        # zero the horizontal halo pads once (tiles are reused; the image
        # region is rewritten every tile, pads stay zero)
        for t_ in lum_ring + sq_ring:
            nc.gpsimd.memset(t_[:, 0:PADL], 0.0)
            nc.gpsimd.memset(t_[:, PADL + W : SW], 0.0)

        def s0_load(t):
            sp = specs[t]
            xin = xin_ring[t % NXIN]
            if sp["b"] < 0:  # combined bottom tiles: both images, 48 rows each
                for b in range(PER_CORE_B):
                    for c in range(C):
                        nc.sync.dma_start(
                            out=xin[48 * b : 48 * b + 48, c * W : (c + 1) * W],
                            in_=x[b, c, sp["r0"] : sp["r0"] + 48, :],
                        )
                return
            for c in range(C):
                # during pipeline fill, split across both HWDGE queues
                eng = nc.scalar if (t < 2 and c == 1) else nc.sync
                eng.dma_start(
                    out=xin[0 : sp["nr"], c * W : (c + 1) * W],
                    in_=x[sp["b"], c, sp["r0"] : sp["r0"] + sp["nr"], :],
                )

        def s0c_conv(t):
            # all 3 channels -> f16 in one wide ACT op; both channel adds
            # then run in the DVE 2x packed-f16 mode
            if t < 2:
                return  # fill fast path adds in f32 straight from xin
            sp = specs[t]
            nr = sp["nr"]
            xin = xin_ring[t % NXIN]
            nc.scalar.activation(
                xc_ring[t % NL01][0:nr, :], xin[0:nr, :], Copy
            )

        def s1_l01(t):
            # adds on DVE, not Pool: concurrent Pool(gpsimd) activity slows
            # DVE scans ~2x (measured), and DVE is the critical engine
            sp = specs[t]
            nr = sp["nr"]
            if t < 2:
                xin = xin_ring[t % NXIN]
                nc.vector.tensor_tensor(
                    l01f_ring[t][0:nr, :],
                    xin[0:nr, 0:W],
                    xin[0:nr, W : 2 * W],
                    ADD,
                )
                return
            xc = xc_ring[t % NL01]
            nc.vector.tensor_tensor(
                l01_ring[t % NL01][0:nr, :], xc[0:nr, 0:W], xc[0:nr, W : 2 * W], ADD
            )

        def s2_lum(t):
            sp = specs[t]
            nr = sp["nr"]
            if t < 2:
                xin = xin_ring[t % NXIN]
                nc.vector.tensor_tensor(
                    lum_ring[t % NLUM][0:nr, PADL : PADL + W],
                    l01f_ring[t][0:nr, :],
                    xin[0:nr, 2 * W : 3 * W],
                    ADD,
                )
                return
            nc.vector.tensor_tensor(
                lum_ring[t % NLUM][0:nr, PADL : PADL + W],
                l01_ring[t % NL01][0:nr, :],
                xc_ring[t % NL01][0:nr, 2 * W : 3 * W],
                ADD,
            )

        def s3_sq(t):
            sp = specs[t]
            nr = sp["nr"]
            lum = lum_ring[t % NLUM]
            nc.scalar.activation(
                sq_ring[t % NSQ][0:nr, PADL : PADL + W],
                lum[0:nr, PADL : PADL + W],
                Square,
            )

        def s4_scans(t):
            sp = specs[t]
            nr = sp["nr"]
            # out[c] = sum_{v=c+1..c+7} src[v]; anchor is exact because
            # src[0..7] are zero pad. matmul rhs for output col j reads
            # out[j+4] (even offsets)
            for src_ring, nsrc, dst_ring, ndst in (
                (lum_ring, NLUM, h1_ring, NH1),
                (sq_ring, NSQ, h2_ring, NH2),
            ):
                src = src_ring[t % nsrc]
                dst = dst_ring[t % ndst]
                nc.vector.tensor_tensor_scan(
                    out=dst[0:nr, 0:SCAN_N],
                    data0=src[0:nr, 7 : 7 + SCAN_N],
                    data1=src[0:nr, 0:SCAN_N],
                    initial=0.0,
                    op0=ADD,
                    op1=SUB,
                )

        S1s, S2s = {}, {}

        def s5_mm1(t):
            sp = specs[t]
            K, M, w = sp["K"], sp["M"], sp["w"]
            h1 = h1_ring[t % NH1]
            S1 = p1pool.tile([128, W], f32, tag="S1", name=f"S1_{t}")
            for half in range(2):
                nc.tensor.matmul(
                    S1[0:M, 512 * half : 512 * half + 512],
                    WB[0:K, 128 * w : 128 * w + M],
                    h1[0:K, 4 + 512 * half : 516 + 512 * half],
                    start=True,
                    stop=True,
                )
            S1s[t] = S1

        def s6_m2(t):
            sp = specs[t]
            M = sp["M"]
            S1 = S1s.pop(t)
            if sp["b"] < 0:  # drain tile: halves overlap the PE chain
                for h in range(2):
                    nc.scalar.activation(
                        m2_ring[t % NM2][0:M, 512 * h : 512 * h + 512],
                        S1[0:M, 512 * h : 512 * h + 512],
                        Square,
                        scale=inv147,
                    )
                return
            nc.scalar.activation(
                m2_ring[t % NM2][0:M, :], S1[0:M, :], Square, scale=inv147
            )

        def s7_mm2(t):
            sp = specs[t]
            K, M, w = sp["K"], sp["M"], sp["w"]
            h2 = h2_ring[t % NH2]
            m2 = m2_ring[t % NM2]
            S2 = p2pool.tile([128, W], f32, tag="S2", name=f"S2_{t}")
            if sp["b"] < 0:  # drain tile: band+I per half, so V h0 can start early
                for half in range(2):
                    c0 = 512 * half
                    nc.tensor.matmul(
                        S2[0:M, c0 : c0 + 512],
                        WBB[0:K, 128 * w : 128 * w + M],
                        h2[0:K, 4 + c0 : 516 + c0],
                        start=True,
                        stop=False,
                        skip_group_check=True,
                    )
                    nc.tensor.matmul(
                        S2[0:M, c0 : c0 + 512],
                        WBB[0:M, 512 : 512 + M],
                        m2[0:M, c0 : c0 + 512],
                        start=False,
                        stop=True,
                        skip_group_check=True,
                    )
                S2s[t] = S2
                return
            for half in range(2):
                c0 = 512 * half
                nc.tensor.matmul(
                    S2[0:M, c0 : c0 + 512],
                    WBB[0:K, 128 * w : 128 * w + M],
                    h2[0:K, 4 + c0 : 516 + c0],
                    start=True,
                    stop=False,
                    skip_group_check=True,
                )
            for half in range(2):
                c0 = 512 * half
                nc.tensor.matmul(
                    S2[0:M, c0 : c0 + 512],
                    WBB[0:M, 512 : 512 + M],
                    m2[0:M, c0 : c0 + 512],
                    start=False,
                    stop=True,
                    skip_group_check=True,
                )
            S2s[t] = S2

        def s8_v(t):
            sp = specs[t]
            M = sp["M"]
            S2 = S2s.pop(t)
            if sp["b"] < 0:
                for h in range(2):
                    nc.scalar.activation(
                        v_ring[t % NV][0:M, 512 * h : 512 * h + 512],
                        S2[0:M, 512 * h : 512 * h + 512],
                        Copy,
                        scale=inv441,
                    )
                return
            nc.scalar.activation(
                v_ring[t % NV][0:M, :], S2[0:M, :], Copy, scale=inv441
            )

        def s9_store(t):
            # SWDGE: software descriptors spread the store across all 16 SDMA
            # engines; an HWDGE store ring pins to 2 engines (22.5 GB/s each).
            sp = specs[t]
            M = sp["M"]
            if sp["b"] < 0:
                # tiny final stores: HWDGE (sync+scalar) beats SWDGE generation
                for b in range(PER_CORE_B):
                    eng = nc.sync if b == 0 else nc.scalar
                    eng.dma_start(
                        out=y[b, 0, sp["out_r0"] : sp["out_r0"] + 45, :],
                        in_=v_ring[t % NV][45 * b : 45 * b + 45, :],
                    )
                return
            nc.gpsimd.dma_start(
                out=y[sp["b"], 0, sp["out_r0"] : sp["out_r0"] + M, :],
                in_=v_ring[t % NV][0:M, :],
            )

        for i in range(NT + 8):
            if 7 <= i < NT + 7:
                s9_store(i - 7)
            if i < NT:
                s0_load(i)
            if 6 <= i < NT + 6:
                s7_mm2(i - 6)
            if 6 <= i < NT + 6:
                s8_v(i - 6)
            if 5 <= i < NT + 5:
                s6_m2(i - 5)
            if 3 <= i < NT + 3:
                s3_sq(i - 3)
            if 4 <= i < NT + 4:
                s4_scans(i - 4)
            if 4 <= i < NT + 4:
                s5_mm1(i - 4)
            if 2 <= i < NT + 2:
                s1_l01(i - 2)
            if 2 <= i < NT + 2:
                s2_lum(i - 2)
            if 1 <= i < NT + 1:
                s0c_conv(i - 1)

    if finalize:
        nc.finalize()
    return nc


def kernel(x, kernel_size):
    assert int(kernel_size) == K7
    x = np.ascontiguousarray(np.asarray(x, dtype=np.float32))
    B = x.shape[0]
    assert x.shape == (B, C, H, W) and B == PER_CORE_B * N_CORES

    from concourse.bass_utils import run_bass_kernel_spmd

    nc = build_nc()
    wb, wbb = band_weights()
    in_maps = [
        {"x": x[i * PER_CORE_B : (i + 1) * PER_CORE_B], "wb": wb, "wbb": wbb}
        for i in range(N_CORES)
    ]
    res = run_bass_kernel_spmd(nc, in_maps, list(range(N_CORES)))
    y = np.concatenate(
        [np.asarray(res.results[i]["y"], dtype=np.float32) for i in range(N_CORES)],
        axis=0,
    )
    return y


# revision 35
# speedup vs baseline: 1.0633x; 1.0136x over previous
"""LocalVarianceMap Trainium2 kernel.

reference:
  lum  = mean over channel of x            (B,1,H,W)
  mean = 7x7 'same' box mean of lum ; sqm = same of lum^2
  out  = sqm - mean^2

Full input x: (16, 3, 1024, 1024) fp32. Data-parallel over batch:
8 NeuronCores x 2 images each. HBM traffic/core = 33.6 MB -> ~93 us
at the 360 GB/s DMA-bus share, which is the target wall.

Per-core pipeline per 128-row tile (partition=h, free=w). Let
s = x0+x1+x2 (channel sum, = 3*lum); all intermediate tiles fp16:

  SP  : 3 HWDGE loads (x0,x1,x2 rows)           [4KB packets]
  Pool: l01 = x0+x1                (fp32)
  DVE : s   = l01+x2               (fp16, stt)
  ACT : sq  = s^2                  (fp16)
  DVE : h1  = 7-col sliding sum of s  (one tensor_tensor_scan, fp32
        carried state, fp16 out), h2 = same of sq
  PE  : S1  = Wband^T @ h1  (2 fp16 matmuls, vertical 7-band sum)
  ACT : m2  = Square(S1/147)       (fp16) == mean^2
  PE  : S2  = Wband^T @ h2 - 441*I @ m2  (4 fp16 matmuls, PSUM accum)
  ACT : V   = Copy(S2/441)         (fp32) == var
  SP  : HWDGE store V

fp16 is safe here: the scan keeps its running state in fp32 (only the
stored outputs round), and all other fp16 roundings are per-element
(bounded, no accumulation across the row).

Vertical tiling overlaps input tiles by 6 rows so each output tile's
vertical halo lives inside one K<=128 banded matmul.
"""

import sys

if "/opt/trn_rl_repo" not in sys.path:
    sys.path.insert(0, "/opt/trn_rl_repo")

import numpy as np
from contextlib import ExitStack

import concourse.bass as bass
import concourse.bacc as bacc
import concourse.tile as tile
from concourse import mybir

H = 1024
W = 1024
C = 3
PER_CORE_B = 2
N_CORES = 8
K7 = 7
PADL = 8                 # left zero pad of s/sq tiles (scan telescoping);
                         # even, so packed-f16 writes at PADL stay 4B-aligned
SW = PADL + W + 4        # fp16 row-tile width (1036; used cols 0..1034)
SCAN_N = W + 4           # even scan width (N=1028): the DVE scan runs 1 cyc/elem
                         # at even N>512, 2 cyc/elem otherwise; h[1028] is unused
HW_ = 1024               # scan data width

f32 = mybir.dt.float32
f16 = mybir.dt.float16
bf16 = mybir.dt.bfloat16
ADD = mybir.AluOpType.add
SUB = mybir.AluOpType.subtract
MUL = mybir.AluOpType.mult
Square = mybir.ActivationFunctionType.Square
Copy = mybir.ActivationFunctionType.Copy


def _tiles():
    # last 48-row tiles of both images share one 96-partition tile (a full
    # tile's scans cost the same regardless of row count)
    specs = []
    for b in range(PER_CORE_B):
        specs.append(dict(b=b, r0=0, nr=128, K=128, M=125, out_r0=0, w=0))
        for t in range(1, 8):
            specs.append(
                dict(b=b, r0=122 * t, nr=128, K=128, M=122, out_r0=122 * t + 3, w=1)
            )
    specs.append(dict(b=-1, r0=976, nr=96, K=96, M=90, out_r0=979, w=3))
    return specs


def band_weights() -> tuple[np.ndarray, np.ndarray]:
    """(fp16 [128,512]: W0|Wmid|Wlast|Wcomb for the mean path,
    bf16 [128,640]: same four bands | -441*I for the squares path)."""
    wb = np.zeros((128, 5 * 128), np.float32)
    for m in range(125):
        for k in range(max(m - 3, 0), m + 4):
            wb[k, m] = 1.0
    for m in range(122):
        for k in range(m, m + 7):
            wb[k, 128 + m] = 1.0
    for m in range(45):
        for k in range(m, min(m + 7, 48)):
            wb[k, 256 + m] = 1.0
    # block 3: both images' bottom-edge tiles, block-diagonal on partitions
    for m in range(45):
        for k in range(m, min(m + 7, 48)):
            wb[k, 384 + m] = 1.0
            wb[48 + k, 384 + 45 + m] = 1.0
    for m in range(128):
        wb[m, 512 + m] = -441.0
    import ml_dtypes

    return wb[:, 0:512].astype(np.float16), wb.astype(ml_dtypes.bfloat16)


def build_nc(finalize: bool = True) -> bass.Bass:
    nc = bacc.Bacc("TRN2", target_bir_lowering=False)

    x = nc.dram_tensor("x", [PER_CORE_B, C, H, W], f32, kind="ExternalInput")
    wbt = nc.dram_tensor("wb", [128, 4 * 128], f16, kind="ExternalInput")
    wbbt = nc.dram_tensor("wbb", [128, 5 * 128], bf16, kind="ExternalInput")
    y = nc.dram_tensor("y", [PER_CORE_B, 1, H, W], f16, kind="ExternalOutput")

    inv147 = float(np.float32(1.0) / np.float32(147.0))
    inv441 = float(np.float32(1.0) / np.float32(441.0))

    specs = _tiles()
    NT = len(specs)

    NXIN, NL01, NLUM, NSQ, NH1, NH2, NM2, NV = 6, 4, 4, 4, 3, 4, 3, 3

    with tile.TileContext(nc) as tc, ExitStack() as ctx:
        cpool = ctx.enter_context(tc.tile_pool(name="const", bufs=1))
        p1pool = ctx.enter_context(tc.tile_pool(name="ps1", bufs=2, space="PSUM"))
        p2pool = ctx.enter_context(tc.tile_pool(name="ps2", bufs=2, space="PSUM"))

        WB = cpool.tile([128, 4 * 128], f16)
        nc.sync.dma_start(out=WB[:], in_=wbt[:, :])
        WBB = cpool.tile([128, 5 * 128], bf16)
        nc.sync.dma_start(out=WBB[:], in_=wbbt[:, :])

        def ring(n, shape, dt, nm):
            return [cpool.tile(shape, dt, tag=f"{nm}{i}", name=f"{nm}{i}") for i in range(n)]

        xin_ring = ring(NXIN, [128, 3 * W], f32, "xin")
        l01_ring = ring(NL01, [128, W], f16, "l01")
        xc_ring = ring(NL01, [128, 3 * W], f16, "xc")
        l01f_ring = ring(2, [128, W], f32, "l01f")  # fill fast path only
        lum_ring = ring(NLUM, [128, SW], f16, "lum")
        # squares path in bf16: lum^2 underflows into f16 subnormals
        # (~0.4% of elements), which stall the DVE scan ~2x; bf16 has no
        # reachable subnormal range here.
        sq_ring = ring(NSQ, [128, SW], bf16, "sq")
        h1_ring = ring(NH1, [128, SCAN_N + 4], f16, "h1")
        h2_ring = ring(NH2, [128, SCAN_N + 4], bf16, "h2")
        m2_ring = ring(NM2, [128, W], bf16, "m2")
        v_ring = ring(NV, [128, W], f16, "v")  # f16 store halves DMA bytes; ~2.4e-4 rel rounding

        # zero the horizontal halo pads once (tiles are reused; the image
        # region is rewritten every tile, pads stay zero)
        for t_ in lum_ring + sq_ring:
            nc.gpsimd.memset(t_[:, 0:PADL], 0.0)
            nc.gpsimd.memset(t_[:, PADL + W : SW], 0.0)

        def s0_load(t):
            sp = specs[t]
            xin = xin_ring[t % NXIN]
            if sp["b"] < 0:  # combined bottom tiles: both images, 48 rows each
                for b in range(PER_CORE_B):
                    for c in range(C):
                        nc.sync.dma_start(
                            out=xin[48 * b : 48 * b + 48, c * W : (c + 1) * W],
                            in_=x[b, c, sp["r0"] : sp["r0"] + 48, :],
                        )
                return
            for c in range(C):
                # during pipeline fill, split across both HWDGE queues
                eng = nc.scalar if (t < 2 and c == 1) else nc.sync
                eng.dma_start(
                    out=xin[0 : sp["nr"], c * W : (c + 1) * W],
                    in_=x[sp["b"], c, sp["r0"] : sp["r0"] + sp["nr"], :],
                )

        def s0c_conv(t):
            # all 3 channels -> f16 in one wide ACT op; both channel adds
            # then run in the DVE 2x packed-f16 mode
            if t < 1:
                return  # fill fast path adds in f32 straight from xin
            sp = specs[t]
            nr = sp["nr"]
            xin = xin_ring[t % NXIN]
            nc.scalar.activation(
                xc_ring[t % NL01][0:nr, :], xin[0:nr, :], Copy
            )

        def s1_l01(t):
            # adds on DVE, not Pool: concurrent Pool(gpsimd) activity slows
            # DVE scans ~2x (measured), and DVE is the critical engine
            sp = specs[t]
            nr = sp["nr"]
            if t < 1:
                xin = xin_ring[t % NXIN]
                nc.vector.tensor_tensor(
                    l01f_ring[t][0:nr, :],
                    xin[0:nr, 0:W],
                    xin[0:nr, W : 2 * W],
                    ADD,
                )
                return
            xc = xc_ring[t % NL01]
            nc.vector.tensor_tensor(
                l01_ring[t % NL01][0:nr, :], xc[0:nr, 0:W], xc[0:nr, W : 2 * W], ADD
            )

        def s2_lum(t):
            sp = specs[t]
            nr = sp["nr"]
            if t < 1:
                xin = xin_ring[t % NXIN]
                nc.vector.tensor_tensor(
                    lum_ring[t % NLUM][0:nr, PADL : PADL + W],
                    l01f_ring[t][0:nr, :],
                    xin[0:nr, 2 * W : 3 * W],
                    ADD,
                )
                return
            nc.vector.tensor_tensor(
                lum_ring[t % NLUM][0:nr, PADL : PADL + W],
                l01_ring[t % NL01][0:nr, :],
                xc_ring[t % NL01][0:nr, 2 * W : 3 * W],
                ADD,
            )

        def s3_sq(t):
            sp = specs[t]
            nr = sp["nr"]
            lum = lum_ring[t % NLUM]
            nc.scalar.activation(
                sq_ring[t % NSQ][0:nr, PADL : PADL + W],
                lum[0:nr, PADL : PADL + W],
                Square,
            )

        def s4_scans(t):
            sp = specs[t]
            nr = sp["nr"]
            # out[c] = sum_{v=c+1..c+7} src[v]; anchor is exact because
            # src[0..7] are zero pad. matmul rhs for output col j reads
            # out[j+4] (even offsets)
            for src_ring, nsrc, dst_ring, ndst in (
                (lum_ring, NLUM, h1_ring, NH1),
                (sq_ring, NSQ, h2_ring, NH2),
            ):
                src = src_ring[t % nsrc]
                dst = dst_ring[t % ndst]
                nc.vector.tensor_tensor_scan(
                    out=dst[0:nr, 0:SCAN_N],
                    data0=src[0:nr, 7 : 7 + SCAN_N],
                    data1=src[0:nr, 0:SCAN_N],
                    initial=0.0,
                    op0=ADD,
                    op1=SUB,
                )

        S1s, S2s = {}, {}

        def s5_mm1(t):
            sp = specs[t]
            K, M, w = sp["K"], sp["M"], sp["w"]
            h1 = h1_ring[t % NH1]
            S1 = p1pool.tile([128, W], f32, tag="S1", name=f"S1_{t}")
            for half in range(2):
                nc.tensor.matmul(
                    S1[0:M, 512 * half : 512 * half + 512],
                    WB[0:K, 128 * w : 128 * w + M],
                    h1[0:K, 4 + 512 * half : 516 + 512 * half],
                    start=True,
                    stop=True,
                )
            S1s[t] = S1

        def s6_m2(t):
            sp = specs[t]
            M = sp["M"]
            S1 = S1s.pop(t)
            if sp["b"] < 0:  # drain tile: halves overlap the PE chain
                for h in range(2):
                    nc.scalar.activation(
                        m2_ring[t % NM2][0:M, 512 * h : 512 * h + 512],
                        S1[0:M, 512 * h : 512 * h + 512],
                        Square,
                        scale=inv147,
                    )
                return
            nc.scalar.activation(
                m2_ring[t % NM2][0:M, :], S1[0:M, :], Square, scale=inv147
            )

        def s7_mm2(t):
            sp = specs[t]
            K, M, w = sp["K"], sp["M"], sp["w"]
            h2 = h2_ring[t % NH2]
            m2 = m2_ring[t % NM2]
            S2 = p2pool.tile([128, W], f32, tag="S2", name=f"S2_{t}")
            if sp["b"] < 0:  # drain tile: band+I per half, so V h0 can start early
                for half in range(2):
                    c0 = 512 * half
                    nc.tensor.matmul(
                        S2[0:M, c0 : c0 + 512],
                        WBB[0:K, 128 * w : 128 * w + M],
                        h2[0:K, 4 + c0 : 516 + c0],
                        start=True,
                        stop=False,
                        skip_group_check=True,
                    )
                    nc.tensor.matmul(
                        S2[0:M, c0 : c0 + 512],
                        WBB[0:M, 512 : 512 + M],
                        m2[0:M, c0 : c0 + 512],
                        start=False,
                        stop=True,
                        skip_group_check=True,
                    )
                S2s[t] = S2
                return
            for half in range(2):
                c0 = 512 * half
                nc.tensor.matmul(
                    S2[0:M, c0 : c0 + 512],
                    WBB[0:K, 128 * w : 128 * w + M],
                    h2[0:K, 4 + c0 : 516 + c0],
                    start=True,
                    stop=False,
                    skip_group_check=True,
                )
            for half in range(2):
                c0 = 512 * half
                nc.tensor.matmul(
                    S2[0:M, c0 : c0 + 512],
                    WBB[0:M, 512 : 512 + M],
                    m2[0:M, c0 : c0 + 512],
                    start=False,
                    stop=True,
                    skip_group_check=True,
                )
            S2s[t] = S2

        def s8_v(t):
            sp = specs[t]
            M = sp["M"]
            S2 = S2s.pop(t)
            if sp["b"] < 0:
                for h in range(2):
                    nc.scalar.activation(
                        v_ring[t % NV][0:M, 512 * h : 512 * h + 512],
                        S2[0:M, 512 * h : 512 * h + 512],
                        Copy,
                        scale=inv441,
                    )
                return
            nc.scalar.activation(
                v_ring[t % NV][0:M, :], S2[0:M, :], Copy, scale=inv441
            )

        def s9_store(t):
            # SWDGE: software descriptors spread the store across all 16 SDMA
            # engines; an HWDGE store ring pins to 2 engines (22.5 GB/s each).
            sp = specs[t]
            M = sp["M"]
            if sp["b"] < 0:
                # tiny final stores: HWDGE (sync+scalar) beats SWDGE generation
                for b in range(PER_CORE_B):
                    eng = nc.sync if b == 0 else nc.scalar
                    eng.dma_start(
                        out=y[b, 0, sp["out_r0"] : sp["out_r0"] + 45, :],
                        in_=v_ring[t % NV][45 * b : 45 * b + 45, :],
                    )
                return
            nc.gpsimd.dma_start(
                out=y[sp["b"], 0, sp["out_r0"] : sp["out_r0"] + M, :],
                in_=v_ring[t % NV][0:M, :],
            )

        for i in range(NT + 8):
            if 7 <= i < NT + 7:
                s9_store(i - 7)
            if i < NT:
                s0_load(i)
            if 6 <= i < NT + 6:
                s7_mm2(i - 6)
            if 6 <= i < NT + 6:
                s8_v(i - 6)
            if 5 <= i < NT + 5:
                s6_m2(i - 5)
            if 3 <= i < NT + 3:
                s3_sq(i - 3)
            if 4 <= i < NT + 4:
                s4_scans(i - 4)
            if 4 <= i < NT + 4:
                s5_mm1(i - 4)
            if 2 <= i < NT + 2:
                s1_l01(i - 2)
            if 2 <= i < NT + 2:
                s2_lum(i - 2)
            if 1 <= i < NT + 1:
                s0c_conv(i - 1)

    if finalize:
        nc.finalize()
    return nc


def kernel(x, kernel_size):
    assert int(kernel_size) == K7
    x = np.ascontiguousarray(np.asarray(x, dtype=np.float32))
    B = x.shape[0]
    assert x.shape == (B, C, H, W) and B == PER_CORE_B * N_CORES

    from concourse.bass_utils import run_bass_kernel_spmd

    nc = build_nc()
    wb, wbb = band_weights()
    in_maps = [
        {"x": x[i * PER_CORE_B : (i + 1) * PER_CORE_B], "wb": wb, "wbb": wbb}
        for i in range(N_CORES)
    ]
    res = run_bass_kernel_spmd(nc, in_maps, list(range(N_CORES)))
    y = np.concatenate(
        [np.asarray(res.results[i]["y"], dtype=np.float32) for i in range(N_CORES)],
        axis=0,
    )
    return y
